# revision 24
# baseline (speedup 1.0000x reference)
"""Trainium2 Bass kernel for nn_DiscreteLoss (data-parallel over batch).

Contract: kernel(**inputs) takes the FULL unsharded inputs (B=64) and
returns the FULL scalar loss.  Internally the batch dim is sharded over
8 NeuronCores (8 batches each); each core produces per-partition partial
sums for every loss term, which the host combines in float64.

Device-side strategy per core (v2, fp8):
  - all bulk tensors ship as fp8-e4m3 (half the HBM bytes of bf16); the
    loss is a statistical sum over ~1M squares per core, so the fp8
    quantization bias (~0.1%) is far below the 2e-2 gate.
  - the mapping gather AND the ground-truth subtraction run in a single
    DoubleRow fp8 matmul per (batch, tensor-pack): stationary weights
    are [E_b ; -I] (host-built one-hot + negated identity, exact in
    fp8), the moving operand is [cur ; gt] stacked along the k-subtile
    axis.  Products are 1.0 * value, so fp32 PSUM results are exact
    over the fp8-rounded inputs.
  - every loss normalizer is folded into host-side sqrt-weight scaling
    (x4 rz/zs, /16 masks, sqrt(2) landmark columns, best-block column
    weights), so the device only computes plain sums of squares:
    ScalarE activation(Square, accum_out) and VectorE
    tensor_tensor_reduce split the PSUM square-reduction work; the tail
    batches are squared per-bank for a short critical path.
  - the KL term uses ln(V*q + V*eps) = ln V + ln(q + eps): one ScalarE
    Ln pass with scale=V, then one VectorE tensor_tensor_reduce.
  - the best-block runs on the otherwise-idle GpSimd engine.
  - 9 DMAs total: 3 small (weights/qy/best, scalar ring) + 5 data
    chunks (sync ring, sized so the last arrivals have the lightest
    post-processing) + 1 store.
"""

import contextlib
import ctypes
import os
import sys
import types

for _p in ("/opt/trn_rl_repo", "/root/.axon_site/_ro/trn_rl_repo"):
    if os.path.isdir(_p) and _p not in sys.path:
        sys.path.append(_p)

import numpy as np

# --- problem constants (hardcoded per spec) ---
B, S, N, D, V = 64, 128, 128, 512, 128
N_CORES = 8
BPC = B // N_CORES          # batches per core = 8
ALPHA, BETA, GAMMA, EPS = 1.0, 0.1, 1.0, 1e-20
MARK = (0, 29, 88, 117)
CB = 1.0 / 8192.0           # common normalizer for the best block

_CACHE = {}


def _install_ntff_hook_shim():
    """run_bass_kernel_spmd(trace=True) looks for antenv.axon_hooks, which
    this image lacks; recreate the ctypes hook against libaxon_pjrt.so."""
    if "antenv.axon_hooks" in sys.modules:
        return
    so_path = "/opt/axon/libaxon_pjrt.so"

    def _get_hook():
        if not os.path.exists(so_path):
            return None
        lib = ctypes.CDLL(so_path)
        if not hasattr(lib, "axon_start_nrt_profile"):
            return None
        lib.axon_start_nrt_profile.argtypes = [
            ctypes.POINTER(ctypes.c_int64), ctypes.c_size_t]
        lib.axon_start_nrt_profile.restype = ctypes.c_int64
        lib.axon_stop_nrt_profile.argtypes = [ctypes.c_char_p]
        lib.axon_stop_nrt_profile.restype = ctypes.c_int64

        @contextlib.contextmanager
        def _hook(output_dir, device_ids):
            import jax
            jax.devices()
            if device_ids:
                ids = (ctypes.c_int64 * len(device_ids))(*device_ids)
                rc = lib.axon_start_nrt_profile(ids, len(device_ids))
            else:
                rc = lib.axon_start_nrt_profile(None, 0)
            if rc != 0:
                raise RuntimeError(f"axon_start_nrt_profile rc={rc}")
            try:
                yield
            finally:
                n = lib.axon_stop_nrt_profile(str(output_dir).encode())
                if n < 0:
                    raise RuntimeError(f"axon_stop_nrt_profile rc={n}")

        return _hook

    mod = types.ModuleType("antenv.axon_hooks")
    mod.get_axon_ntff_profile_hook = _get_hook
    mod.set_axon_ntff_profile_hook = lambda h: None
    sys.modules["antenv.axon_hooks"] = mod


# per-batch data chunks alternate between the two HWDGE rings so batches
# arrive every ~0.7us; weights lead on the sync ring
CHUNKS = tuple((b, 1, "scalar" if b % 2 == 0 else "sync") for b in range(8))
# DoubleRow fp8 matmul folds gather + gt-subtract into one MM; flip off to
# fall back to two accumulating plain-fp8 MMs per (batch, pack).
USE_DOUBLE_ROW = True


def _build_program():
    import concourse.bacc as bacc
    import concourse.tile as tile
    from concourse import mybir

    f32 = mybir.dt.float32
    bf16 = mybir.dt.bfloat16
    fp8 = mybir.dt.float8e4
    nc = bacc.Bacc(None, target_bir_lowering=False, debug=False)

    DR = mybir.MatmulPerfMode.DoubleRow
    SQUARE = mybir.ActivationFunctionType.Square
    LN = mybir.ActivationFunctionType.Ln
    AL = mybir.AluOpType

    # ---- per-core DRAM parameters (host pre-packed, fp8) ----
    # data[j, b, t, c, d]: t=0 -> rz pack (cur=4*rzs, gt=4*zs)
    #                      t=1 -> pm pack (cur=[pts*w|masks/16], gt likewise)
    d_data = nc.declare_dram_parameter("data", [128, BPC, 2, 2, 512], fp8,
                                       isOutput=False)
    # w[j, b, k, s]: k=0 -> one-hot E_b, k=1 -> -I
    d_w = nc.declare_dram_parameter("w", [128, BPC, 2, 128], fp8, isOutput=False)
    d_qy = nc.declare_dram_parameter("qy", [128, BPC * V], bf16, isOutput=False)
    # best[p, 0:64] = weighted X block, [64:128] = weighted Y block
    d_best = nc.declare_dram_parameter("best", [128, 128], bf16, isOutput=False)
    # partial sums: [0,0] main total, col 1 kld, col 2 best
    d_out = nc.declare_dram_parameter("acc", [128, 3], f32, isOutput=True)

    with tile.TileContext(nc) as tc:
        with contextlib.ExitStack() as ctx:
            singles = ctx.enter_context(tc.tile_pool(name="singles", bufs=1))
            junka = ctx.enter_context(tc.tile_pool(name="junka", bufs=3))
            junkv = ctx.enter_context(tc.tile_pool(name="junkv", bufs=2))
            psp = ctx.enter_context(tc.tile_pool(name="ps", bufs=3, space="PSUM"))
            psps = ctx.enter_context(tc.tile_pool(name="pss", bufs=1, space="PSUM"))

            # ---- weights first on the sync ring (every MM gates on them);
            # batch-0 data streams in parallel on the scalar ring, then
            # qy/best; remaining chunks follow on the sync ring ----
            t_w = singles.tile([128, BPC, 2, 128], fp8)
            nc.sync.dma_start(out=t_w[:], in_=d_w.ap())

            t_qy = singles.tile([128, BPC * V], bf16)
            t_best = singles.tile([128, 128], bf16)
            d_tiles = []
            for b0, nb, ring in CHUNKS:
                t = singles.tile([128, nb, 2, 2, 512], fp8, tag=f"d{b0}")
                eng = nc.scalar if ring == "scalar" else nc.sync
                eng.dma_start(out=t[:], in_=d_data.ap()[:, b0:b0 + nb])
                d_tiles.append(t)
                if b0 == 2:      # qy mid-stream: its Ln+kld chain hides
                    nc.scalar.dma_start(out=t_qy[:], in_=d_qy.ap())
            nc.scalar.dma_start(out=t_best[:], in_=d_best.ap())

            # [0,0] main total, col 1 kld, col 2 best
            acc = singles.tile([128, 3], f32)

            # ---- KL chain: Ln(V*q + V*eps) = ln V + ln(q+eps) ----
            # (tensor_tensor_reduce is a custom-DVE op that faults the HW on
            # this runtime path; tensor_scalar's fused accumulate is safe.)
            t_eps = singles.tile([128, 1], f32)
            nc.vector.memset(t_eps[:], float(V) * EPS)
            lnq = singles.tile([128, BPC * V], bf16)
            nc.scalar.activation(out=lnq[:], in_=t_qy[:], func=LN,
                                 bias=t_eps[:], scale=float(V))
            jk = junkv.tile([128, BPC * V], bf16, tag="jv")
            nc.vector.tensor_mul(jk[:], t_qy[:], lnq[:])
            jk2 = junkv.tile([128, BPC * V], bf16, tag="jv")
            nc.vector.tensor_scalar(out=jk2[:], in0=jk[:], scalar1=1.0,
                                    scalar2=None, op0=AL.mult, op1=AL.add,
                                    accum_out=acc[:, 1:2])

            # ---- best block (tiny; DVE sub + square + fused reduce) ----
            dbest = singles.tile([128, 64], bf16)
            nc.vector.tensor_sub(dbest[:], t_best[:, 0:64], t_best[:, 64:128])
            jb = singles.tile([128, 64], bf16, tag="jb")
            nc.vector.tensor_mul(jb[:], dbest[:], dbest[:])
            jb2 = singles.tile([128, 64], bf16, tag="jb2")
            nc.vector.tensor_scalar(out=jb2[:], in0=jb[:], scalar1=1.0,
                                    scalar2=None, op0=AL.mult, op1=AL.add,
                                    accum_out=acc[:, 2:3])

            # ---- per-batch: 2 DoubleRow MMs -> [128,1024] diff tile ->
            # square to SBUF bf16 (ACT Square, or DVE cast+mul for offload
            # batches) -> ones-vector matmul accumulates the column sums of
            # every batch's squares into one [1,1024] PSUM group, so the
            # final reduction is a single DVE pass instead of 8.
            DVE_BATCHES = (3, 6)
            t_ones = singles.tile([128, 1], bf16)
            nc.vector.memset(t_ones[:], 1.0)

            ci_of = {}
            for ci, (b0, nb, _r) in enumerate(CHUNKS):
                for i in range(nb):
                    ci_of[b0 + i] = (ci, i)

            sum_ps = psps.tile([1, 1024], f32, tag="sum")

            # PE warm-up: the HAM clock gate holds the PE at 1.2 GHz until
            # it has been busy ~3.4us. Burn that window on dummy matmuls
            # while the first data chunks stream, so real MMs run at 2.4.
            warm = singles.tile([128, 512], bf16, tag="warm")
            nc.gpsimd.memset(warm[:], 1.0)
            for _ in range(6):
                nc.tensor.matmul(sum_ps[:, 0:512], lhsT=t_ones[:],
                                 rhs=warm[:], start=True, stop=True,
                                 skip_group_check=True)
            def emit_sum(b, sq):
                # accumulate batch b's column sums into the shared group
                for h in range(2):
                    nc.tensor.matmul(
                        sum_ps[:, h * 512:(h + 1) * 512], lhsT=t_ones[:],
                        rhs=sq[:, h * 512:(h + 1) * 512],
                        start=(b == 0), stop=(b == BPC - 1),
                        skip_group_check=True)

            # sum-MMs are emitted one batch late so the PE's in-order queue
            # never stalls on the square that feeds them.
            sq_tiles = {}
            for b in range(BPC):
                ci, i = ci_of[b]
                ps = psp.tile([128, 1024], f32, tag="g")
                for tpack in range(2):  # 0 = rz, 1 = pm
                    dst = ps[:, tpack * 512:(tpack + 1) * 512]
                    if USE_DOUBLE_ROW:
                        nc.tensor.matmul(
                            dst, lhsT=t_w[:, b, :, :],
                            rhs=d_tiles[ci][:, i, tpack, :, :],
                            start=True, stop=True, perf_mode=DR)
                    else:
                        nc.tensor.matmul(
                            dst, lhsT=t_w[:, b, 0, :],
                            rhs=d_tiles[ci][:, i, tpack, 0, :],
                            start=True, stop=False)
                        nc.tensor.matmul(
                            dst, lhsT=t_w[:, b, 1, :],
                            rhs=d_tiles[ci][:, i, tpack, 1, :],
                            start=False, stop=True)
                if b > 0:
                    emit_sum(b - 1, sq_tiles.pop(b - 1))
                if b in DVE_BATCHES:
                    jc = junkv.tile([128, 1024], bf16, tag="jv")
                    nc.vector.tensor_copy(jc[:], ps[:])
                    sq = junkv.tile([128, 1024], bf16, tag="jm")
                    nc.vector.tensor_mul(sq[:], jc[:], jc[:])
                else:
                    sq = junka.tile([128, 1024], bf16, tag="ja")
                    nc.scalar.activation(out=sq[:], in_=ps[:], func=SQUARE)
                sq_tiles[b] = sq
            emit_sum(BPC - 1, sq_tiles.pop(BPC - 1))

            # one final fused reduce of the [1,1024] column sums
            jf = junkv.tile([1, 1024], bf16, tag="jf")
            nc.vector.tensor_scalar(out=jf[:], in0=sum_ps[:], scalar1=1.0,
                                    scalar2=None, op0=AL.mult, op1=AL.add,
                                    accum_out=acc[0:1, 0:1])

            # ---- store partial sums ----
            nc.sync.dma_start(out=d_out.ap(), in_=acc[:])

    nc.compile()
    # insert_act_table_loads emits a redundant exp_and_others (set 0) load on
    # top of natural_log (set 5), which covers every func used here; drop it
    # to save ~1.3us of ScalarE time.
    import concourse.mybir as _mb
    for _b in nc.main_func.blocks:
        _loads = [i for i in _b.instructions
                  if isinstance(i, _mb.InstLoadActFuncSet)]
        if len(_loads) > 1 and any(l.act_func_set_id == 5 for l in _loads):
            for _l in _loads:
                if _l.act_func_set_id != 5:
                    _b.instructions.remove(_l)
    return nc


def _get_program():
    if "nc" not in _CACHE:
        _CACHE["nc"] = _build_program()
    return _CACHE["nc"]


def _shard_inputs(inputs):
    """Pack the full B=64 inputs into 8 per-core fp8 input maps."""
    from concourse import mybir
    fp8 = mybir.dt.np(mybir.dt.float8e4)
    import ml_dtypes
    bf16 = ml_dtypes.bfloat16

    f = lambda k: np.asarray(inputs[k], dtype=np.float32)

    # -- bulk data pack [B, 128(j), 2, 2, 512] --
    wpts = np.ones(N, dtype=np.float32)
    for n in MARK:
        wpts[n] = np.sqrt(np.float32(2.0))
    big = np.empty((B, 128, 2, 2, 512), dtype=np.float32)
    big[:, :, 0, 0] = f("rzs") * np.float32(4.0)
    big[:, :, 0, 1] = f("zs") * np.float32(4.0)
    msc = np.float32(1.0 / 16.0)
    big[:, :, 1, 0, 0:256] = (f("pts") * wpts[None, None, :, None]).reshape(B, S, 256)
    big[:, :, 1, 0, 256:512] = f("masks").reshape(B, S, 256) * msc
    big[:, :, 1, 1, 0:256] = (f("pts_gt") * wpts[None, None, :, None]).reshape(B, S, 256)
    big[:, :, 1, 1, 256:512] = f("masks_gt").reshape(B, S, 256) * msc
    big = np.clip(big, -240.0, 240.0).astype(fp8)

    # -- DoubleRow weights [B, 128(j), 2, 128(s)]: [E_b ; -I] --
    mapping = np.asarray(inputs["mapping"]).astype(np.int32)       # [B, S]
    iota = np.arange(128, dtype=np.int32)
    wfull = np.zeros((B, 128, 2, 128), dtype=np.float32)
    wfull[:, :, 0, :] = (mapping[:, None, :] == iota[None, :, None])
    wfull[:, :, 1, :] = -np.eye(128, dtype=np.float32)[None]
    wfull = wfull.astype(fp8)

    qy = f("qy")                                                    # [B, S, V]

    # -- best block [8, 128, 128] bf16: weighted X | Y --
    wb = np.full(N, 1.0 / 128.0, dtype=np.float64)
    for n in MARK:
        wb[n] = np.sqrt(1.0 + 8192.0 / (2.0 * (B * N) ** 2))
    wb = wb.astype(np.float32)
    cm = np.zeros(N, dtype=np.float32)
    cm[32:96] = 1.0
    best = np.empty((N_CORES, 128, 128), dtype=np.float32)
    for side, rz_k, pt_k, mk_k in (
            (0, "best_rz", "best_pt", "best_mask"),
            (64, "logits", "best_pt_gt", "best_mask_gt")):
        best[:, :, side + 0:side + 32] = (
            f(rz_k) * np.float32(0.5)).reshape(N_CORES, 128, 32)
        best[:, :, side + 32:side + 48] = (
            f(pt_k) * wb[None, :, None]).reshape(N_CORES, 128, 16)
        best[:, :, side + 48:side + 64] = (
            f(mk_k) * cm[None, :, None]).reshape(N_CORES, 128, 16)
    best = best.astype(bf16)

    in_maps = []
    for c in range(N_CORES):
        lo, hi = c * BPC, (c + 1) * BPC
        m = {
            "data": np.ascontiguousarray(big[lo:hi].transpose(1, 0, 2, 3, 4)),
            "w": np.ascontiguousarray(wfull[lo:hi].transpose(1, 0, 2, 3)),
            "qy": np.ascontiguousarray(
                qy[lo:hi].transpose(1, 0, 2).reshape(128, BPC * V)).astype(bf16),
            "best": np.ascontiguousarray(best[c]),
        }
        in_maps.append(m)
    return in_maps


def _combine(results):
    """Host-side float64 reduction of the per-core partial sums."""
    s_main = s_kld = s_best = 0.0
    for r in results:
        a = r["acc"].astype(np.float64)
        s_main += a[0, 0]
        s_kld += a[:, 1].sum()
        s_best += a[:, 2].sum()
    ret = (s_best * CB + s_main / (B * S) + BETA * s_kld / (B * S))
    return np.float32(ret * B)


def run_sharded(inputs, trace=False):
    """Compile (cached), run on the 8 cores, return (scalar, BassKernelResults)."""
    _install_ntff_hook_shim()
    from concourse.bass_utils import run_bass_kernel_spmd

    assert int(inputs["vector_dims"]) == V
    nc = _get_program()
    in_maps = _shard_inputs(inputs)
    res = run_bass_kernel_spmd(nc, in_maps, list(range(N_CORES)), trace=trace)
    return _combine(res.results), res


def kernel(**inputs) -> np.ndarray:
    out, _ = run_sharded(inputs, trace=False)
    return out


# revision 25
# speedup vs baseline: 1.0483x; 1.0483x over previous
"""Trainium2 Bass kernel for nn_DiscreteLoss (data-parallel over batch).

Contract: kernel(**inputs) takes the FULL unsharded inputs (B=64) and
returns the FULL scalar loss.  Internally the batch dim is sharded over
8 NeuronCores (8 batches each); each core produces per-partition partial
sums for every loss term, which the host combines in float64.

Device-side strategy per core (v2, fp8):
  - all bulk tensors ship as fp8-e4m3 (half the HBM bytes of bf16); the
    loss is a statistical sum over ~1M squares per core, so the fp8
    quantization bias (~0.1%) is far below the 2e-2 gate.
  - the mapping gather AND the ground-truth subtraction run in a single
    DoubleRow fp8 matmul per (batch, tensor-pack): stationary weights
    are [E_b ; -I] (host-built one-hot + negated identity, exact in
    fp8), the moving operand is [cur ; gt] stacked along the k-subtile
    axis.  Products are 1.0 * value, so fp32 PSUM results are exact
    over the fp8-rounded inputs.
  - every loss normalizer is folded into host-side sqrt-weight scaling
    (x4 rz/zs, /16 masks, sqrt(2) landmark columns, best-block column
    weights), so the device only computes plain sums of squares:
    ScalarE activation(Square, accum_out) and VectorE
    tensor_tensor_reduce split the PSUM square-reduction work; the tail
    batches are squared per-bank for a short critical path.
  - the KL term uses ln(V*q + V*eps) = ln V + ln(q + eps): one ScalarE
    Ln pass with scale=V, then one VectorE tensor_tensor_reduce.
  - the best-block runs on the otherwise-idle GpSimd engine.
  - 9 DMAs total: 3 small (weights/qy/best, scalar ring) + 5 data
    chunks (sync ring, sized so the last arrivals have the lightest
    post-processing) + 1 store.
"""

import contextlib
import ctypes
import os
import sys
import types

for _p in ("/opt/trn_rl_repo", "/root/.axon_site/_ro/trn_rl_repo"):
    if os.path.isdir(_p) and _p not in sys.path:
        sys.path.append(_p)

import numpy as np

# --- problem constants (hardcoded per spec) ---
B, S, N, D, V = 64, 128, 128, 512, 128
N_CORES = 8
BPC = B // N_CORES          # batches per core = 8
ALPHA, BETA, GAMMA, EPS = 1.0, 0.1, 1.0, 1e-20
MARK = (0, 29, 88, 117)
CB = 1.0 / 8192.0           # common normalizer for the best block

_CACHE = {}


def _install_ntff_hook_shim():
    """run_bass_kernel_spmd(trace=True) looks for antenv.axon_hooks, which
    this image lacks; recreate the ctypes hook against libaxon_pjrt.so."""
    if "antenv.axon_hooks" in sys.modules:
        return
    so_path = "/opt/axon/libaxon_pjrt.so"

    def _get_hook():
        if not os.path.exists(so_path):
            return None
        lib = ctypes.CDLL(so_path)
        if not hasattr(lib, "axon_start_nrt_profile"):
            return None
        lib.axon_start_nrt_profile.argtypes = [
            ctypes.POINTER(ctypes.c_int64), ctypes.c_size_t]
        lib.axon_start_nrt_profile.restype = ctypes.c_int64
        lib.axon_stop_nrt_profile.argtypes = [ctypes.c_char_p]
        lib.axon_stop_nrt_profile.restype = ctypes.c_int64

        @contextlib.contextmanager
        def _hook(output_dir, device_ids):
            import jax
            jax.devices()
            if device_ids:
                ids = (ctypes.c_int64 * len(device_ids))(*device_ids)
                rc = lib.axon_start_nrt_profile(ids, len(device_ids))
            else:
                rc = lib.axon_start_nrt_profile(None, 0)
            if rc != 0:
                raise RuntimeError(f"axon_start_nrt_profile rc={rc}")
            try:
                yield
            finally:
                n = lib.axon_stop_nrt_profile(str(output_dir).encode())
                if n < 0:
                    raise RuntimeError(f"axon_stop_nrt_profile rc={n}")

        return _hook

    mod = types.ModuleType("antenv.axon_hooks")
    mod.get_axon_ntff_profile_hook = _get_hook
    mod.set_axon_ntff_profile_hook = lambda h: None
    sys.modules["antenv.axon_hooks"] = mod


# data chunks: batch 0 on the scalar ring (parallel with the weights on
# the sync ring), the rest on the sync ring in batch order
CHUNKS = ((0, 1, "scalar"), (1, 1, "sync"), (2, 2, "sync"), (4, 2, "sync"),
          (6, 1, "sync"), (7, 1, "sync"))
# DoubleRow fp8 matmul folds gather + gt-subtract into one MM; flip off to
# fall back to two accumulating plain-fp8 MMs per (batch, pack).
USE_DOUBLE_ROW = True


def _build_program():
    import concourse.bacc as bacc
    import concourse.tile as tile
    from concourse import mybir

    f32 = mybir.dt.float32
    bf16 = mybir.dt.bfloat16
    fp8 = mybir.dt.float8e4
    nc = bacc.Bacc(None, target_bir_lowering=False, debug=False)

    DR = mybir.MatmulPerfMode.DoubleRow
    SQUARE = mybir.ActivationFunctionType.Square
    LN = mybir.ActivationFunctionType.Ln
    AL = mybir.AluOpType

    # ---- per-core DRAM parameters (host pre-packed, fp8) ----
    # data[j, b, t, c, d]: t=0 -> rz pack (cur=4*rzs, gt=4*zs)
    #                      t=1 -> pm pack (cur=[pts*w|masks/16], gt likewise)
    d_data = nc.declare_dram_parameter("data", [128, BPC, 2, 2, 512], fp8,
                                       isOutput=False)
    # w[j, b, k, s]: k=0 -> one-hot E_b, k=1 -> -I
    d_w = nc.declare_dram_parameter("w", [128, BPC, 2, 128], fp8, isOutput=False)
    d_qy = nc.declare_dram_parameter("qy", [128, BPC * V], bf16, isOutput=False)
    # best[p, 0:64] = weighted X block, [64:128] = weighted Y block
    d_best = nc.declare_dram_parameter("best", [128, 128], bf16, isOutput=False)
    # partial sums: [0,0] main total, col 1 kld, col 2 best
    d_out = nc.declare_dram_parameter("acc", [128, 3], f32, isOutput=True)

    with tile.TileContext(nc) as tc:
        with contextlib.ExitStack() as ctx:
            singles = ctx.enter_context(tc.tile_pool(name="singles", bufs=1))
            junka = ctx.enter_context(tc.tile_pool(name="junka", bufs=3))
            junkv = ctx.enter_context(tc.tile_pool(name="junkv", bufs=2))
            psp = ctx.enter_context(tc.tile_pool(name="ps", bufs=3, space="PSUM"))
            psps = ctx.enter_context(tc.tile_pool(name="pss", bufs=1, space="PSUM"))

            # ---- weights first on the sync ring (every MM gates on them);
            # batch-0 data streams in parallel on the scalar ring, then
            # qy/best; remaining chunks follow on the sync ring ----
            t_w = singles.tile([128, BPC, 2, 128], fp8)
            nc.sync.dma_start(out=t_w[:], in_=d_w.ap())

            t_qy = singles.tile([128, BPC * V], bf16)
            t_best = singles.tile([128, 128], bf16)
            d_tiles = []
            for b0, nb, ring in CHUNKS:
                t = singles.tile([128, nb, 2, 2, 512], fp8, tag=f"d{b0}")
                eng = nc.scalar if ring == "scalar" else nc.sync
                eng.dma_start(out=t[:], in_=d_data.ap()[:, b0:b0 + nb])
                d_tiles.append(t)
                if b0 == 2:      # qy mid-stream: its Ln+kld chain hides
                    nc.scalar.dma_start(out=t_qy[:], in_=d_qy.ap())
            nc.scalar.dma_start(out=t_best[:], in_=d_best.ap())

            # [0,0] main total, col 1 kld, col 2 best
            acc = singles.tile([128, 3], f32)

            # ---- KL chain: Ln(V*q + V*eps) = ln V + ln(q+eps) ----
            # (tensor_tensor_reduce is a custom-DVE op that faults the HW on
            # this runtime path; tensor_scalar's fused accumulate is safe.)
            t_eps = singles.tile([128, 1], f32)
            nc.vector.memset(t_eps[:], float(V) * EPS)
            lnq = singles.tile([128, BPC * V], bf16)
            nc.scalar.activation(out=lnq[:], in_=t_qy[:], func=LN,
                                 bias=t_eps[:], scale=float(V))
            jk = junkv.tile([128, BPC * V], bf16, tag="jv")
            nc.vector.tensor_mul(jk[:], t_qy[:], lnq[:])
            jk2 = junkv.tile([128, BPC * V], bf16, tag="jv")
            nc.vector.tensor_scalar(out=jk2[:], in0=jk[:], scalar1=1.0,
                                    scalar2=None, op0=AL.mult, op1=AL.add,
                                    accum_out=acc[:, 1:2])

            # ---- best block (tiny; DVE sub + square + fused reduce) ----
            dbest = singles.tile([128, 64], bf16)
            nc.vector.tensor_sub(dbest[:], t_best[:, 0:64], t_best[:, 64:128])
            jb = singles.tile([128, 64], bf16, tag="jb")
            nc.vector.tensor_mul(jb[:], dbest[:], dbest[:])
            jb2 = singles.tile([128, 64], bf16, tag="jb2")
            nc.vector.tensor_scalar(out=jb2[:], in0=jb[:], scalar1=1.0,
                                    scalar2=None, op0=AL.mult, op1=AL.add,
                                    accum_out=acc[:, 2:3])

            # ---- per-batch: 2 DoubleRow MMs -> [128,1024] diff tile ->
            # square to SBUF bf16 (ACT Square, or DVE cast+mul for offload
            # batches) -> ones-vector matmul accumulates the column sums of
            # every batch's squares into one [1,1024] PSUM group, so the
            # final reduction is a single DVE pass instead of 8.
            DVE_BATCHES = (3, 6)
            t_ones = singles.tile([128, 1], bf16)
            nc.vector.memset(t_ones[:], 1.0)

            ci_of = {}
            for ci, (b0, nb, _r) in enumerate(CHUNKS):
                for i in range(nb):
                    ci_of[b0 + i] = (ci, i)

            sum_ps = psps.tile([1, 1024], f32, tag="sum")

            # PE warm-up: the HAM clock gate holds the PE at 1.2 GHz until
            # it has been busy ~3.4us. Burn that window on dummy matmuls
            # while the first data chunks stream, so real MMs run at 2.4.
            warm = singles.tile([128, 512], bf16, tag="warm")
            nc.gpsimd.memset(warm[:], 1.0)
            for _ in range(6):
                nc.tensor.matmul(sum_ps[:, 0:512], lhsT=t_ones[:],
                                 rhs=warm[:], start=True, stop=True,
                                 skip_group_check=True)
            def emit_sum(b, sq):
                # accumulate batch b's column sums into the shared group
                for h in range(2):
                    nc.tensor.matmul(
                        sum_ps[:, h * 512:(h + 1) * 512], lhsT=t_ones[:],
                        rhs=sq[:, h * 512:(h + 1) * 512],
                        start=(b == 0), stop=(b == BPC - 1),
                        skip_group_check=True)

            # sum-MMs are emitted one batch late so the PE's in-order queue
            # never stalls on the square that feeds them.
            sq_tiles = {}
            for b in range(BPC):
                ci, i = ci_of[b]
                ps = psp.tile([128, 1024], f32, tag="g")
                for tpack in range(2):  # 0 = rz, 1 = pm
                    dst = ps[:, tpack * 512:(tpack + 1) * 512]
                    if USE_DOUBLE_ROW:
                        nc.tensor.matmul(
                            dst, lhsT=t_w[:, b, :, :],
                            rhs=d_tiles[ci][:, i, tpack, :, :],
                            start=True, stop=True, perf_mode=DR)
                    else:
                        nc.tensor.matmul(
                            dst, lhsT=t_w[:, b, 0, :],
                            rhs=d_tiles[ci][:, i, tpack, 0, :],
                            start=True, stop=False)
                        nc.tensor.matmul(
                            dst, lhsT=t_w[:, b, 1, :],
                            rhs=d_tiles[ci][:, i, tpack, 1, :],
                            start=False, stop=True)
                if b > 0:
                    emit_sum(b - 1, sq_tiles.pop(b - 1))
                if b in DVE_BATCHES:
                    jc = junkv.tile([128, 1024], bf16, tag="jv")
                    nc.vector.tensor_copy(jc[:], ps[:])
                    sq = junkv.tile([128, 1024], bf16, tag="jm")
                    nc.vector.tensor_mul(sq[:], jc[:], jc[:])
                else:
                    sq = junka.tile([128, 1024], bf16, tag="ja")
                    nc.scalar.activation(out=sq[:], in_=ps[:], func=SQUARE)
                sq_tiles[b] = sq
            emit_sum(BPC - 1, sq_tiles.pop(BPC - 1))

            # one final fused reduce of the [1,1024] column sums
            jf = junkv.tile([1, 1024], bf16, tag="jf")
            nc.vector.tensor_scalar(out=jf[:], in0=sum_ps[:], scalar1=1.0,
                                    scalar2=None, op0=AL.mult, op1=AL.add,
                                    accum_out=acc[0:1, 0:1])

            # ---- store partial sums ----
            nc.sync.dma_start(out=d_out.ap(), in_=acc[:])

    nc.compile()
    # insert_act_table_loads emits a redundant exp_and_others (set 0) load on
    # top of natural_log (set 5), which covers every func used here; drop it
    # to save ~1.3us of ScalarE time.
    import concourse.mybir as _mb
    for _b in nc.main_func.blocks:
        _loads = [i for i in _b.instructions
                  if isinstance(i, _mb.InstLoadActFuncSet)]
        if len(_loads) > 1 and any(l.act_func_set_id == 5 for l in _loads):
            for _l in _loads:
                if _l.act_func_set_id != 5:
                    _b.instructions.remove(_l)
    return nc


def _get_program():
    if "nc" not in _CACHE:
        _CACHE["nc"] = _build_program()
    return _CACHE["nc"]


def _shard_inputs(inputs):
    """Pack the full B=64 inputs into 8 per-core fp8 input maps."""
    from concourse import mybir
    fp8 = mybir.dt.np(mybir.dt.float8e4)
    import ml_dtypes
    bf16 = ml_dtypes.bfloat16

    f = lambda k: np.asarray(inputs[k], dtype=np.float32)

    # -- bulk data pack [B, 128(j), 2, 2, 512] --
    wpts = np.ones(N, dtype=np.float32)
    for n in MARK:
        wpts[n] = np.sqrt(np.float32(2.0))
    big = np.empty((B, 128, 2, 2, 512), dtype=np.float32)
    big[:, :, 0, 0] = f("rzs") * np.float32(4.0)
    big[:, :, 0, 1] = f("zs") * np.float32(4.0)
    msc = np.float32(1.0 / 16.0)
    big[:, :, 1, 0, 0:256] = (f("pts") * wpts[None, None, :, None]).reshape(B, S, 256)
    big[:, :, 1, 0, 256:512] = f("masks").reshape(B, S, 256) * msc
    big[:, :, 1, 1, 0:256] = (f("pts_gt") * wpts[None, None, :, None]).reshape(B, S, 256)
    big[:, :, 1, 1, 256:512] = f("masks_gt").reshape(B, S, 256) * msc
    big = np.clip(big, -240.0, 240.0).astype(fp8)

    # -- DoubleRow weights [B, 128(j), 2, 128(s)]: [E_b ; -I] --
    mapping = np.asarray(inputs["mapping"]).astype(np.int32)       # [B, S]
    iota = np.arange(128, dtype=np.int32)
    wfull = np.zeros((B, 128, 2, 128), dtype=np.float32)
    wfull[:, :, 0, :] = (mapping[:, None, :] == iota[None, :, None])
    wfull[:, :, 1, :] = -np.eye(128, dtype=np.float32)[None]
    wfull = wfull.astype(fp8)

    qy = f("qy")                                                    # [B, S, V]

    # -- best block [8, 128, 128] bf16: weighted X | Y --
    wb = np.full(N, 1.0 / 128.0, dtype=np.float64)
    for n in MARK:
        wb[n] = np.sqrt(1.0 + 8192.0 / (2.0 * (B * N) ** 2))
    wb = wb.astype(np.float32)
    cm = np.zeros(N, dtype=np.float32)
    cm[32:96] = 1.0
    best = np.empty((N_CORES, 128, 128), dtype=np.float32)
    for side, rz_k, pt_k, mk_k in (
            (0, "best_rz", "best_pt", "best_mask"),
            (64, "logits", "best_pt_gt", "best_mask_gt")):
        best[:, :, side + 0:side + 32] = (
            f(rz_k) * np.float32(0.5)).reshape(N_CORES, 128, 32)
        best[:, :, side + 32:side + 48] = (
            f(pt_k) * wb[None, :, None]).reshape(N_CORES, 128, 16)
        best[:, :, side + 48:side + 64] = (
            f(mk_k) * cm[None, :, None]).reshape(N_CORES, 128, 16)
    best = best.astype(bf16)

    in_maps = []
    for c in range(N_CORES):
        lo, hi = c * BPC, (c + 1) * BPC
        m = {
            "data": np.ascontiguousarray(big[lo:hi].transpose(1, 0, 2, 3, 4)),
            "w": np.ascontiguousarray(wfull[lo:hi].transpose(1, 0, 2, 3)),
            "qy": np.ascontiguousarray(
                qy[lo:hi].transpose(1, 0, 2).reshape(128, BPC * V)).astype(bf16),
            "best": np.ascontiguousarray(best[c]),
        }
        in_maps.append(m)
    return in_maps


def _combine(results):
    """Host-side float64 reduction of the per-core partial sums."""
    s_main = s_kld = s_best = 0.0
    for r in results:
        a = r["acc"].astype(np.float64)
        s_main += a[0, 0]
        s_kld += a[:, 1].sum()
        s_best += a[:, 2].sum()
    ret = (s_best * CB + s_main / (B * S) + BETA * s_kld / (B * S))
    return np.float32(ret * B)


def run_sharded(inputs, trace=False):
    """Compile (cached), run on the 8 cores, return (scalar, BassKernelResults)."""
    _install_ntff_hook_shim()
    from concourse.bass_utils import run_bass_kernel_spmd

    assert int(inputs["vector_dims"]) == V
    nc = _get_program()
    in_maps = _shard_inputs(inputs)
    res = run_bass_kernel_spmd(nc, in_maps, list(range(N_CORES)), trace=trace)
    return _combine(res.results), res


def kernel(**inputs) -> np.ndarray:
    out, _ = run_sharded(inputs, trace=False)
    return out


# revision 27
# speedup vs baseline: 1.0764x; 1.0268x over previous
"""Trainium2 Bass kernel for nn_DiscreteLoss (data-parallel over batch).

Contract: kernel(**inputs) takes the FULL unsharded inputs (B=64) and
returns the FULL scalar loss.  Internally the batch dim is sharded over
8 NeuronCores (8 batches each); each core produces per-partition partial
sums for every loss term, which the host combines in float64.

Device-side strategy per core (v2, fp8):
  - all bulk tensors ship as fp8-e4m3 (half the HBM bytes of bf16); the
    loss is a statistical sum over ~1M squares per core, so the fp8
    quantization bias (~0.1%) is far below the 2e-2 gate.
  - the mapping gather AND the ground-truth subtraction run in a single
    DoubleRow fp8 matmul per (batch, tensor-pack): stationary weights
    are [E_b ; -I] (host-built one-hot + negated identity, exact in
    fp8), the moving operand is [cur ; gt] stacked along the k-subtile
    axis.  Products are 1.0 * value, so fp32 PSUM results are exact
    over the fp8-rounded inputs.
  - every loss normalizer is folded into host-side sqrt-weight scaling
    (x4 rz/zs, /16 masks, sqrt(2) landmark columns, best-block column
    weights), so the device only computes plain sums of squares:
    ScalarE activation(Square, accum_out) and VectorE
    tensor_tensor_reduce split the PSUM square-reduction work; the tail
    batches are squared per-bank for a short critical path.
  - the KL term uses ln(V*q + V*eps) = ln V + ln(q + eps): one ScalarE
    Ln pass with scale=V, then one VectorE tensor_tensor_reduce.
  - the best-block runs on the otherwise-idle GpSimd engine.
  - 9 DMAs total: 3 small (weights/qy/best, scalar ring) + 5 data
    chunks (sync ring, sized so the last arrivals have the lightest
    post-processing) + 1 store.
"""

import contextlib
import ctypes
import os
import sys
import types

for _p in ("/opt/trn_rl_repo", "/root/.axon_site/_ro/trn_rl_repo"):
    if os.path.isdir(_p) and _p not in sys.path:
        sys.path.append(_p)

import numpy as np

# --- problem constants (hardcoded per spec) ---
B, S, N, D, V = 64, 128, 128, 512, 128
N_CORES = 8
BPC = B // N_CORES          # batches per core = 8
ALPHA, BETA, GAMMA, EPS = 1.0, 0.1, 1.0, 1e-20
MARK = (0, 29, 88, 117)
CB = 1.0 / 8192.0           # common normalizer for the best block

_CACHE = {}


def _install_ntff_hook_shim():
    """run_bass_kernel_spmd(trace=True) looks for antenv.axon_hooks, which
    this image lacks; recreate the ctypes hook against libaxon_pjrt.so."""
    if "antenv.axon_hooks" in sys.modules:
        return
    so_path = "/opt/axon/libaxon_pjrt.so"

    def _get_hook():
        if not os.path.exists(so_path):
            return None
        lib = ctypes.CDLL(so_path)
        if not hasattr(lib, "axon_start_nrt_profile"):
            return None
        lib.axon_start_nrt_profile.argtypes = [
            ctypes.POINTER(ctypes.c_int64), ctypes.c_size_t]
        lib.axon_start_nrt_profile.restype = ctypes.c_int64
        lib.axon_stop_nrt_profile.argtypes = [ctypes.c_char_p]
        lib.axon_stop_nrt_profile.restype = ctypes.c_int64

        @contextlib.contextmanager
        def _hook(output_dir, device_ids):
            import jax
            jax.devices()
            if device_ids:
                ids = (ctypes.c_int64 * len(device_ids))(*device_ids)
                rc = lib.axon_start_nrt_profile(ids, len(device_ids))
            else:
                rc = lib.axon_start_nrt_profile(None, 0)
            if rc != 0:
                raise RuntimeError(f"axon_start_nrt_profile rc={rc}")
            try:
                yield
            finally:
                n = lib.axon_stop_nrt_profile(str(output_dir).encode())
                if n < 0:
                    raise RuntimeError(f"axon_stop_nrt_profile rc={n}")

        return _hook

    mod = types.ModuleType("antenv.axon_hooks")
    mod.get_axon_ntff_profile_hook = _get_hook
    mod.set_axon_ntff_profile_hook = lambda h: None
    sys.modules["antenv.axon_hooks"] = mod


# data chunks: batch 0 on the scalar ring (parallel with the weights on
# the sync ring), the rest on the sync ring in batch order
CHUNKS = ((0, 1, "scalar"), (1, 1, "sync"), (2, 2, "sync"), (4, 2, "sync"),
          (6, 1, "sync"), (7, 1, "sync"))
# DoubleRow fp8 matmul folds gather + gt-subtract into one MM; flip off to
# fall back to two accumulating plain-fp8 MMs per (batch, pack).
USE_DOUBLE_ROW = True


def _build_program():
    import concourse.bacc as bacc
    import concourse.tile as tile
    from concourse import mybir

    f32 = mybir.dt.float32
    bf16 = mybir.dt.bfloat16
    fp8 = mybir.dt.float8e4
    nc = bacc.Bacc(None, target_bir_lowering=False, debug=False)

    DR = mybir.MatmulPerfMode.DoubleRow
    SQUARE = mybir.ActivationFunctionType.Square
    LN = mybir.ActivationFunctionType.Ln
    AL = mybir.AluOpType

    # ---- per-core DRAM parameters (host pre-packed, fp8) ----
    # data[j, b, t, c, d]: t=0 -> rz pack (cur=4*rzs, gt=4*zs)
    #                      t=1 -> pm pack (cur=[pts*w|masks/16], gt likewise)
    d_data = nc.declare_dram_parameter("data", [128, BPC, 2, 2, 512], fp8,
                                       isOutput=False)
    # w[j, b, k, s]: k=0 -> one-hot E_b, k=1 -> -I
    d_w = nc.declare_dram_parameter("w", [128, BPC, 2, 128], fp8, isOutput=False)
    d_qy = nc.declare_dram_parameter("qy", [128, BPC * V], bf16, isOutput=False)
    # best[p, 0:64] = weighted X block, [64:128] = weighted Y block
    d_best = nc.declare_dram_parameter("best", [128, 128], bf16, isOutput=False)
    # partial sums: [0,0] main(sum-batches), col1 kld, col2 best, col3/4 b5/b7
    d_out = nc.declare_dram_parameter("acc", [128, 5], f32, isOutput=True)

    with tile.TileContext(nc) as tc:
        with contextlib.ExitStack() as ctx:
            singles = ctx.enter_context(tc.tile_pool(name="singles", bufs=1))
            junka = ctx.enter_context(tc.tile_pool(name="junka", bufs=3))
            junkv = ctx.enter_context(tc.tile_pool(name="junkv", bufs=2))
            psp = ctx.enter_context(tc.tile_pool(name="ps", bufs=3, space="PSUM"))
            psps = ctx.enter_context(tc.tile_pool(name="pss", bufs=1, space="PSUM"))

            # ---- weights first on the sync ring (every MM gates on them);
            # batch-0 data streams in parallel on the scalar ring, then
            # qy/best; remaining chunks follow on the sync ring ----
            t_w = singles.tile([128, BPC, 2, 128], fp8)
            nc.sync.dma_start(out=t_w[:], in_=d_w.ap())

            t_qy = singles.tile([128, BPC * V], bf16)
            t_best = singles.tile([128, 128], bf16)
            d_tiles = []
            for b0, nb, ring in CHUNKS:
                t = singles.tile([128, nb, 2, 2, 512], fp8, tag=f"d{b0}")
                eng = nc.scalar if ring == "scalar" else nc.sync
                eng.dma_start(out=t[:], in_=d_data.ap()[:, b0:b0 + nb])
                d_tiles.append(t)
                if b0 == 2:      # qy mid-stream: its Ln+kld chain hides
                    nc.scalar.dma_start(out=t_qy[:], in_=d_qy.ap())
            nc.scalar.dma_start(out=t_best[:], in_=d_best.ap())

            # [0,0] main(sum-batches), col1 kld, col2 best, col3/4 b5/b7
            acc = singles.tile([128, 5], f32)

            # ---- KL chain: Ln(V*q + V*eps) = ln V + ln(q+eps) ----
            # (tensor_tensor_reduce is a custom-DVE op that faults the HW on
            # this runtime path; tensor_scalar's fused accumulate is safe.)
            t_eps = singles.tile([128, 1], f32)
            nc.vector.memset(t_eps[:], float(V) * EPS)
            lnq = singles.tile([128, BPC * V], bf16)
            nc.scalar.activation(out=lnq[:], in_=t_qy[:], func=LN,
                                 bias=t_eps[:], scale=float(V))
            jk = junkv.tile([128, BPC * V], bf16, tag="jv")
            nc.vector.tensor_mul(jk[:], t_qy[:], lnq[:])
            jk2 = junkv.tile([128, BPC * V], bf16, tag="jv")
            nc.vector.tensor_scalar(out=jk2[:], in0=jk[:], scalar1=1.0,
                                    scalar2=None, op0=AL.mult, op1=AL.add,
                                    accum_out=acc[:, 1:2])

            # ---- best block (tiny; DVE sub + square + fused reduce) ----
            dbest = singles.tile([128, 64], bf16)
            nc.vector.tensor_sub(dbest[:], t_best[:, 0:64], t_best[:, 64:128])
            jb = singles.tile([128, 64], bf16, tag="jb")
            nc.vector.tensor_mul(jb[:], dbest[:], dbest[:])
            jb2 = singles.tile([128, 64], bf16, tag="jb2")
            nc.vector.tensor_scalar(out=jb2[:], in0=jb[:], scalar1=1.0,
                                    scalar2=None, op0=AL.mult, op1=AL.add,
                                    accum_out=acc[:, 2:3])

            # ---- per-batch: 2 DoubleRow MMs -> [128,1024] diff tile ->
            # square to SBUF bf16 (ACT Square, or DVE cast+mul for offload
            # batches) -> ones-vector matmul accumulates the column sums of
            # every batch's squares into one [1,1024] PSUM group, so the
            # final reduction is a single DVE pass instead of 8.
            DVE_BATCHES = (3, 6)
            t_ones = singles.tile([128, 1], bf16)
            nc.vector.memset(t_ones[:], 1.0)

            ci_of = {}
            for ci, (b0, nb, _r) in enumerate(CHUNKS):
                for i in range(nb):
                    ci_of[b0 + i] = (ci, i)

            # single-bank [1,512] accumulator: both halves of every batch's
            # column sums pile into the same 512 columns
            sum_ps = psps.tile([1, 512], f32, tag="sum")
            # batches whose reduce goes through ACT's accumulator directly
            # (the stream tail: keeps sum-MMs off the PE's critical path)
            ACC_BATCHES = (5, 7)
            SUM_BATCHES = tuple(b for b in range(BPC) if b not in ACC_BATCHES)

            # PE warm-up: the HAM clock gate holds the PE at 1.2 GHz until
            # it has been busy ~3.4us. Burn that window on dummy matmuls
            # while the first data chunks stream, so real MMs run at 2.4.
            warm = singles.tile([128, 512], bf16, tag="warm")
            nc.gpsimd.memset(warm[:], 1.0)
            for _ in range(6):
                nc.tensor.matmul(sum_ps[:], lhsT=t_ones[:],
                                 rhs=warm[:], start=True, stop=True,
                                 skip_group_check=True)

            def emit_sum(b, sq):
                # accumulate batch b's column sums into the shared group
                for h in range(2):
                    nc.tensor.matmul(
                        sum_ps[:], lhsT=t_ones[:],
                        rhs=sq[:, h * 512:(h + 1) * 512],
                        start=(b == SUM_BATCHES[0] and h == 0),
                        stop=(b == SUM_BATCHES[-1] and h == 1),
                        skip_group_check=True)

            # sum-MMs are emitted one batch late so the PE's in-order queue
            # never stalls on the square that feeds them.
            sq_tiles = {}
            for b in range(BPC):
                ci, i = ci_of[b]
                ps = psp.tile([128, 1024], f32, tag="g")
                for tpack in range(2):  # 0 = rz, 1 = pm
                    dst = ps[:, tpack * 512:(tpack + 1) * 512]
                    if USE_DOUBLE_ROW:
                        nc.tensor.matmul(
                            dst, lhsT=t_w[:, b, :, :],
                            rhs=d_tiles[ci][:, i, tpack, :, :],
                            start=True, stop=True, perf_mode=DR)
                    else:
                        nc.tensor.matmul(
                            dst, lhsT=t_w[:, b, 0, :],
                            rhs=d_tiles[ci][:, i, tpack, 0, :],
                            start=True, stop=False)
                        nc.tensor.matmul(
                            dst, lhsT=t_w[:, b, 1, :],
                            rhs=d_tiles[ci][:, i, tpack, 1, :],
                            start=False, stop=True)
                if b > 0 and (b - 1) in sq_tiles:
                    emit_sum(b - 1, sq_tiles.pop(b - 1))
                if b in ACC_BATCHES:
                    # direct square + accumulator reduce on ACT
                    ja = junka.tile([128, 1024], bf16, tag="ja")
                    nc.scalar.activation(out=ja[:], in_=ps[:], func=SQUARE,
                                         accum_out=acc[:, 3 + ACC_BATCHES.index(b):
                                                       4 + ACC_BATCHES.index(b)])
                    continue
                if b in DVE_BATCHES:
                    jc = junkv.tile([128, 1024], bf16, tag="jv")
                    nc.vector.tensor_copy(jc[:], ps[:])
                    sq = junkv.tile([128, 1024], bf16, tag="jm")
                    nc.vector.tensor_mul(sq[:], jc[:], jc[:])
                else:
                    sq = junka.tile([128, 1024], bf16, tag="ja")
                    nc.scalar.activation(out=sq[:], in_=ps[:], func=SQUARE)
                sq_tiles[b] = sq
            for b in sorted(sq_tiles):
                emit_sum(b, sq_tiles[b])
            sq_tiles.clear()

            # one final fused reduce of the [1,512] column sums
            jf = junkv.tile([1, 512], bf16, tag="jf")
            nc.vector.tensor_scalar(out=jf[:], in0=sum_ps[:], scalar1=1.0,
                                    scalar2=None, op0=AL.mult, op1=AL.add,
                                    accum_out=acc[0:1, 0:1])

            # ---- store partial sums ----
            nc.sync.dma_start(out=d_out.ap(), in_=acc[:])

    nc.compile()
    # insert_act_table_loads emits a redundant exp_and_others (set 0) load on
    # top of natural_log (set 5), which covers every func used here; drop it
    # to save ~1.3us of ScalarE time.
    import concourse.mybir as _mb
    for _b in nc.main_func.blocks:
        _loads = [i for i in _b.instructions
                  if isinstance(i, _mb.InstLoadActFuncSet)]
        if len(_loads) > 1 and any(l.act_func_set_id == 5 for l in _loads):
            for _l in _loads:
                if _l.act_func_set_id != 5:
                    _b.instructions.remove(_l)
    return nc


def _get_program():
    if "nc" not in _CACHE:
        _CACHE["nc"] = _build_program()
    return _CACHE["nc"]


def _shard_inputs(inputs):
    """Pack the full B=64 inputs into 8 per-core fp8 input maps."""
    from concourse import mybir
    fp8 = mybir.dt.np(mybir.dt.float8e4)
    import ml_dtypes
    bf16 = ml_dtypes.bfloat16

    f = lambda k: np.asarray(inputs[k], dtype=np.float32)

    # -- bulk data pack [B, 128(j), 2, 2, 512] --
    wpts = np.ones(N, dtype=np.float32)
    for n in MARK:
        wpts[n] = np.sqrt(np.float32(2.0))
    big = np.empty((B, 128, 2, 2, 512), dtype=np.float32)
    big[:, :, 0, 0] = f("rzs") * np.float32(4.0)
    big[:, :, 0, 1] = f("zs") * np.float32(4.0)
    msc = np.float32(1.0 / 16.0)
    big[:, :, 1, 0, 0:256] = (f("pts") * wpts[None, None, :, None]).reshape(B, S, 256)
    big[:, :, 1, 0, 256:512] = f("masks").reshape(B, S, 256) * msc
    big[:, :, 1, 1, 0:256] = (f("pts_gt") * wpts[None, None, :, None]).reshape(B, S, 256)
    big[:, :, 1, 1, 256:512] = f("masks_gt").reshape(B, S, 256) * msc
    big = np.clip(big, -240.0, 240.0).astype(fp8)

    # -- DoubleRow weights [B, 128(j), 2, 128(s)]: [E_b ; -I] --
    mapping = np.asarray(inputs["mapping"]).astype(np.int32)       # [B, S]
    iota = np.arange(128, dtype=np.int32)
    wfull = np.zeros((B, 128, 2, 128), dtype=np.float32)
    wfull[:, :, 0, :] = (mapping[:, None, :] == iota[None, :, None])
    wfull[:, :, 1, :] = -np.eye(128, dtype=np.float32)[None]
    wfull = wfull.astype(fp8)

    qy = f("qy")                                                    # [B, S, V]

    # -- best block [8, 128, 128] bf16: weighted X | Y --
    wb = np.full(N, 1.0 / 128.0, dtype=np.float64)
    for n in MARK:
        wb[n] = np.sqrt(1.0 + 8192.0 / (2.0 * (B * N) ** 2))
    wb = wb.astype(np.float32)
    cm = np.zeros(N, dtype=np.float32)
    cm[32:96] = 1.0
    best = np.empty((N_CORES, 128, 128), dtype=np.float32)
    for side, rz_k, pt_k, mk_k in (
            (0, "best_rz", "best_pt", "best_mask"),
            (64, "logits", "best_pt_gt", "best_mask_gt")):
        best[:, :, side + 0:side + 32] = (
            f(rz_k) * np.float32(0.5)).reshape(N_CORES, 128, 32)
        best[:, :, side + 32:side + 48] = (
            f(pt_k) * wb[None, :, None]).reshape(N_CORES, 128, 16)
        best[:, :, side + 48:side + 64] = (
            f(mk_k) * cm[None, :, None]).reshape(N_CORES, 128, 16)
    best = best.astype(bf16)

    in_maps = []
    for c in range(N_CORES):
        lo, hi = c * BPC, (c + 1) * BPC
        m = {
            "data": np.ascontiguousarray(big[lo:hi].transpose(1, 0, 2, 3, 4)),
            "w": np.ascontiguousarray(wfull[lo:hi].transpose(1, 0, 2, 3)),
            "qy": np.ascontiguousarray(
                qy[lo:hi].transpose(1, 0, 2).reshape(128, BPC * V)).astype(bf16),
            "best": np.ascontiguousarray(best[c]),
        }
        in_maps.append(m)
    return in_maps


def _combine(results):
    """Host-side float64 reduction of the per-core partial sums."""
    s_main = s_kld = s_best = 0.0
    for r in results:
        a = r["acc"].astype(np.float64)
        s_main += a[0, 0] + a[:, 3].sum() + a[:, 4].sum()
        s_kld += a[:, 1].sum()
        s_best += a[:, 2].sum()
    ret = (s_best * CB + s_main / (B * S) + BETA * s_kld / (B * S))
    return np.float32(ret * B)


def run_sharded(inputs, trace=False):
    """Compile (cached), run on the 8 cores, return (scalar, BassKernelResults)."""
    _install_ntff_hook_shim()
    from concourse.bass_utils import run_bass_kernel_spmd

    assert int(inputs["vector_dims"]) == V
    nc = _get_program()
    in_maps = _shard_inputs(inputs)
    res = run_bass_kernel_spmd(nc, in_maps, list(range(N_CORES)), trace=trace)
    return _combine(res.results), res


def kernel(**inputs) -> np.ndarray:
    out, _ = run_sharded(inputs, trace=False)
    return out
